# revision 25
# baseline (speedup 1.0000x reference)
"""AttentionPooling Trainium2 kernel (8-core data-parallel), v3.

Math per batch row b (B=2048, S=512, D=128):
    score[b,s] = x[b,s,:] . qk + ps[b,s]        (qk = Wk^T q * SCALE)
    out[b]     = (sum_s e_s x_s) @ Wv^T / (sum_s e_s) + bv,   e = exp(score)
ps = pos . qp (+ mask fold to -1e30) is tiny and precomputed host-side
(same O(B*S*4) folding the v1 kernel did).

Device layout (per core, BSH=256 batches): token scramble s = 4p + t, so
partition p holds 4 consecutive tokens of a batch and the x DMA moves
2 KB contiguous DRAM runs (16 queues hit line rate ~360 GB/s; the DMA
floor ~188 us/core is the roofline for this memory-regime problem).

Engine split per 8-batch granule [128p, 8b, 4t, 128d], from measured
per-op rates (microbench.py): DMA ~189 us busy, DVE ~175, ACT ~128,
PE ~33 — DMA-bound as a memory-regime kernel should be.
  DMA   x granule f32 in (2 KB packets, issued from SP); the output
        store is issued from ACT so it never blocks the SP x stream.
  ACT   f32 -> bf16 convert (~4.1 us/granule) + staged exp.
  DVE   scores: bf16 TT-mult against broadcast qk (no STT accumulator —
        its ~460 ns/instr fixed cost killed an earlier version), 4
        strided bf16 halving folds 128->8, segmented tensor_reduce
        8->1 into f32 scores; staged ps-add and L-partial reduce.
  PE    4 bf16 matmuls per batch (stationary x tile, moving e column)
        accumulating T[:,b] PSUM columns; per-block: ones-matmul for L,
        bf16 Wv^T projection, then one DVE STT (proj*(1/L) + bv) and
        the store. Stage sizes ramp 8/16/32 at the stream edges to cut
        pipeline fill/drain.
"""

import numpy as np

TOKEN_DIM = 128
SCALE = TOKEN_DIM ** -0.5
B, S, D = 2048, 512, 128
NCORES = 8
BSH = B // NCORES          # 256 batches per core
TPB = 4                    # tokens per partition per batch (s = 4p + t)
GR = 8                     # batches per granule
NGR = BSH // GR            # 32 granules per core
BLK = 128                  # batches per output block
GPB = BLK // GR            # granules per block (16)
NBLK = BSH // BLK          # 2
SEG = GR * TPB             # 32 score segments per granule
STG = 4                    # granules per exp/L staging group
GSB = STG * GR             # batches per staging group (32)

_CACHE = {}


def _split_multi_waits(nc):
    """The walrus build here rejects instructions carrying more than one
    semaphore wait (limit varies by ISA struct; STT and Drain allow 1).
    Hoist extra waits onto same-engine NoOps placed just before the
    instruction — identical blocking semantics, trivial cost."""
    from concourse import mybir

    n = 0
    for f in nc.m.functions:
        for bb in f.blocks:
            new = []
            for inst in bb.instructions:
                si = inst.sync_info
                if si is not None and si.on_wait and len(si.on_wait) > 1:
                    waits = list(si.on_wait)
                    for w in waits[1:]:
                        n += 1
                        nop = mybir.InstNoOp(
                            name=f"T-wsplit-{n}", engine=inst.engine, ins=[], outs=[]
                        )
                        nop.sync_info = mybir.SyncInfo(on_wait=[w], on_update=[])
                        new.append(nop)
                    inst.sync_info = mybir.SyncInfo(
                        on_wait=[waits[0]], on_update=list(si.on_update or [])
                    )
                new.append(inst)
            bb.instructions = new
    return n


def build_program():
    """Build the per-core Bass program (SPMD across the 8 cores)."""
    import concourse.bass as bass
    import concourse.tile as tile
    from concourse import mybir

    f32 = mybir.dt.float32
    bf16 = mybir.dt.bfloat16
    Exp = mybir.ActivationFunctionType.Exp
    Copy = mybir.ActivationFunctionType.Copy
    Add = mybir.AluOpType.add
    Mult = mybir.AluOpType.mult
    X = mybir.AxisListType.X

    nc = bass.Bass("TRN2", target_bir_lowering=False, debug=False)
    x_d = nc.dram_tensor("x", [BSH, S, D], f32, kind="ExternalInput").ap()
    ps_d = nc.dram_tensor("ps", [128, BSH, TPB], f32, kind="ExternalInput").ap()
    qkr_d = nc.dram_tensor("qkr", [128, TPB * D], bf16, kind="ExternalInput").ap()
    wvt_d = nc.dram_tensor("wvt", [D, D], bf16, kind="ExternalInput").ap()
    bvb_d = nc.dram_tensor("bvb", [128, D], f32, kind="ExternalInput").ap()
    out_d = nc.dram_tensor("out", [BSH, D], f32, kind="ExternalOutput").ap()

    with tile.TileContext(nc) as tc:
        with (
            tc.tile_pool(name="consts", bufs=1) as consts,
            tc.tile_pool(name="xf", bufs=4) as xf_pool,
            tc.tile_pool(name="xb", bufs=6) as xb_pool,
            tc.tile_pool(name="prod", bufs=1) as prod_pool,
            tc.tile_pool(name="f1", bufs=1) as f1_pool,
            tc.tile_pool(name="f2", bufs=1) as f2_pool,
            tc.tile_pool(name="f3", bufs=1) as f3_pool,
            tc.tile_pool(name="f4", bufs=1) as f4_pool,
            tc.tile_pool(name="sc", bufs=2) as sc_pool,
            tc.tile_pool(name="sce", bufs=2) as sce_pool,
            tc.tile_pool(name="e", bufs=2) as e_pool,
            tc.tile_pool(name="P", bufs=2) as P_pool,
            tc.tile_pool(name="tpsum", bufs=2, space="PSUM") as tpsum_pool,
            tc.tile_pool(name="epi_psum", bufs=2, space="PSUM") as epi_psum,
            tc.tile_pool(name="epi", bufs=4) as epi_pool,
        ):
            # Prefetch the first batches as SEPARATE small DMAs before the
            # const burst, so the first convert/score chain starts ~2.5 us in
            # instead of waiting a full 8-batch transfer.
            prefetch = []
            pb0 = 0
            for pgr in (2, 2, 4):
                t_ = xf_pool.tile([128, pgr, TPB, D], f32, tag=f"xfp{pb0}")
                nc.sync.dma_start(
                    t_[:],
                    x_d[pb0 : pb0 + pgr].rearrange("b (p t) d -> p b t d", t=TPB),
                )
                prefetch.append(t_)
                pb0 += pgr
            qkr_sb = consts.tile([128, TPB * D], bf16)
            nc.sync.dma_start(qkr_sb[:], qkr_d[:])
            bvb_sb = consts.tile([128, D], f32)
            nc.sync.dma_start(bvb_sb[:], bvb_d[:])
            ps_sb = consts.tile([128, BSH, TPB], f32)
            nc.sync.dma_start(ps_sb[:], ps_d[:])
            wvt_sb = consts.tile([D, D], bf16)
            nc.sync.dma_start(wvt_sb[:], wvt_d[:])
            ones_sb = consts.tile([128, 1], f32)
            nc.vector.memset(ones_sb[:], 1.0)

            def qk_bcast(gr):
                return qkr_sb[:].rearrange("p (o f) -> p o f", o=1).broadcast_to(
                    [128, gr, TPB * D]
                )

            # Per-block stage plans: each stage is a list of granule sizes
            # (batches). Small stages at the start of block 0 (DVE gets fed
            # early) and at the end of block 1 (short drain after last DMA).
            # Each block's batch total is BLK=128.
            stage_plan = [
                [[2, 2], [4], [8], [8, 8], [8, 8, 8, 8], [8, 8, 8, 8],
                 [8, 8, 8, 8]],
                [[8, 8, 8, 8], [8, 8, 8, 8], [8, 8, 8, 8], [8, 8],
                 [8, 4, 2, 2]],
            ]
            for blk in range(NBLK):
                Tpsum = tpsum_pool.tile([128, BLK], f32)
                P_blk = P_pool.tile([128, BLK], f32)
                bofs = 0          # batch offset within the block
                for stage in stage_plan[blk]:
                    gsb = sum(stage)
                    st_b0 = bofs
                    sc = sc_pool.tile([128, gsb, TPB], f32)
                    parts = []    # (xb tile, gr, batch offset in stage)
                    sofs = 0
                    for gr in stage:
                        b0 = blk * BLK + bofs
                        seg = gr * TPB
                        if prefetch:
                            xf = prefetch.pop(0)
                        else:
                            xf = xf_pool.tile([128, gr, TPB, D], f32)
                            nc.sync.dma_start(
                                xf[:],
                                x_d[b0 : b0 + gr].rearrange(
                                    "b (p t) d -> p b t d", t=TPB
                                ),
                            )
                        xb = xb_pool.tile([128, gr, TPB, D], bf16)
                        nc.scalar.activation(xb[:], xf[:], Copy)
                        parts.append((xb, gr, sofs))

                        # scores: prod = xb*qk (bf16), fold 128 -> 8, reduce
                        prod = prod_pool.tile([128, seg, D], bf16)
                        nc.vector.tensor_tensor(
                            out=prod[:].rearrange("p s d -> p (s d)").rearrange(
                                "p (b f) -> p b f", b=gr
                            ),
                            in0=xb[:].rearrange("p b t d -> p b (t d)"),
                            in1=qk_bcast(gr),
                            op=Mult,
                        )
                        f1 = f1_pool.tile([128, seg, 64], bf16)
                        nc.vector.tensor_tensor(
                            out=f1[:], in0=prod[:, :, 0:64], in1=prod[:, :, 64:128],
                            op=Add,
                        )
                        f2 = f2_pool.tile([128, seg, 32], bf16)
                        nc.vector.tensor_tensor(
                            out=f2[:], in0=f1[:, :, 0:32], in1=f1[:, :, 32:64], op=Add
                        )
                        f3 = f3_pool.tile([128, seg, 16], bf16)
                        nc.vector.tensor_tensor(
                            out=f3[:], in0=f2[:, :, 0:16], in1=f2[:, :, 16:32], op=Add
                        )
                        f4 = f4_pool.tile([128, seg, 8], bf16)
                        nc.vector.tensor_tensor(
                            out=f4[:], in0=f3[:, :, 0:8], in1=f3[:, :, 8:16], op=Add
                        )
                        nc.vector.tensor_reduce(
                            out=sc[:, sofs : sofs + gr, :].rearrange(
                                "p b t -> p (b t)"
                            ),
                            in_=f4[:], axis=X, op=Add,
                        )
                        sofs += gr
                        bofs += gr
                    # staged softmax prep over this stage's batches
                    sb0 = blk * BLK + st_b0
                    sce = sce_pool.tile([128, gsb, TPB], f32)
                    nc.vector.tensor_tensor(
                        out=sce[:], in0=sc[:], in1=ps_sb[:, sb0 : sb0 + gsb, :], op=Add
                    )
                    e = e_pool.tile([128, gsb, TPB], bf16)
                    nc.scalar.activation(e[:], sce[:], Exp)
                    nc.vector.tensor_reduce(
                        out=P_blk[:, st_b0 : st_b0 + gsb], in_=e[:], axis=X, op=Add
                    )
                    for xb, gr, sofs in parts:
                        for j in range(gr):
                            bcol = st_b0 + sofs + j
                            for t in range(TPB):
                                nc.tensor.matmul(
                                    out=Tpsum[:, bcol : bcol + 1],
                                    lhsT=xb[:, j, t, :],
                                    rhs=e[:, sofs + j, t : t + 1],
                                    start=(t == 0),
                                    stop=(t == TPB - 1),
                                )
                # block epilogue
                Lp = epi_psum.tile([128, 1], f32, tag="Lp")
                nc.tensor.matmul(
                    out=Lp[:], lhsT=P_blk[:], rhs=ones_sb[:], start=True, stop=True
                )
                rcpL = epi_pool.tile([128, 1], f32, tag="rcpL")
                nc.vector.reciprocal(rcpL[:], Lp[:])
                Tsb = epi_pool.tile([128, BLK], bf16, tag="Tsb")
                nc.scalar.activation(Tsb[:], Tpsum[:], Copy)
                proj = epi_psum.tile([128, D], f32, tag="proj")
                nc.tensor.matmul(
                    out=proj[:], lhsT=Tsb[:], rhs=wvt_sb[:], start=True, stop=True
                )
                out_sb = epi_pool.tile([128, D], f32, tag="out_sb")
                nc.vector.scalar_tensor_tensor(
                    out=out_sb[:], in0=proj[:], scalar=rcpL[:], in1=bvb_sb[:],
                    op0=Mult, op1=Add,
                )
                # issue from ACT so the SP engine's DMA stream (next block's x
                # granules) is not serialized behind the epilogue chain
                nc.scalar.dma_start(out_d[blk * BLK : (blk + 1) * BLK, :], out_sb[:])

    _split_multi_waits(nc)
    return nc


def prepare_inputs(input_features, positions, mask, query, Wk, bk, Wv, bv, Wp, bp):
    """Host-side prep: shard along batch, replicate/fold the small weights."""
    import ml_dtypes

    q = np.asarray(query, np.float32)[0]
    qk = (q @ np.asarray(Wk, np.float32)) * SCALE           # [D]
    qp = (q @ np.asarray(Wp, np.float32)) * SCALE           # [4]
    qkr = np.ascontiguousarray(
        np.broadcast_to(
            np.tile(qk, TPB)[None, :].astype(ml_dtypes.bfloat16), (128, TPB * D)
        )
    )
    wvt = np.ascontiguousarray(np.asarray(Wv, np.float32).T.astype(ml_dtypes.bfloat16))
    bvb = np.ascontiguousarray(
        np.broadcast_to(np.asarray(bv, np.float32)[None, :], (128, D))
    )

    # ps[b, s] = pos . qp with masked tokens forced to -1e30 so their
    # softmax weight underflows to exactly 0. Packed as [128 p, B, 4 t]
    # matching the device's s = 4p + t token scramble.
    ps = np.asarray(positions, np.float32) @ qp              # [B, S]
    m = np.asarray(mask, bool)
    if not m.all():
        ps = np.where(m, ps, np.float32(-1e30))
    ps = np.ascontiguousarray(
        ps.reshape(B, 128, TPB).transpose(1, 0, 2), np.float32
    )

    x = np.asarray(input_features, np.float32)
    in_maps = []
    for c in range(NCORES):
        in_maps.append(
            {
                "x": x[c * BSH : (c + 1) * BSH],
                "ps": np.ascontiguousarray(ps[:, c * BSH : (c + 1) * BSH]),
                "qkr": qkr,
                "wvt": wvt,
                "bvb": bvb,
            }
        )
    return in_maps


def kernel(input_features, positions, mask, query, Wk, bk, Wv, bv, Wp, bp):
    from concourse.bass_utils import run_bass_kernel_spmd

    if "nc" not in _CACHE:
        _CACHE["nc"] = build_program()
    nc = _CACHE["nc"]
    in_maps = prepare_inputs(
        input_features, positions, mask, query, Wk, bk, Wv, bv, Wp, bp
    )
    res = run_bass_kernel_spmd(nc, in_maps, list(range(NCORES)))
    return np.concatenate([res.results[c]["out"] for c in range(NCORES)], axis=0)


# revision 27
# speedup vs baseline: 1.1014x; 1.1014x over previous
"""AttentionPooling Trainium2 kernel (8-core data-parallel), v3.

Math per batch row b (B=2048, S=512, D=128):
    score[b,s] = x[b,s,:] . qk + ps[b,s]        (qk = Wk^T q * SCALE)
    out[b]     = (sum_s e_s x_s) @ Wv^T / (sum_s e_s) + bv,   e = exp(score)
ps = pos . qp (+ mask fold to -1e30) is tiny and precomputed host-side
(same O(B*S*4) folding the v1 kernel did).

Device layout (per core, BSH=256 batches): token scramble s = 4p + t, so
partition p holds 4 consecutive tokens of a batch and the x DMA moves
2 KB contiguous DRAM runs (16 queues hit line rate ~360 GB/s; the DMA
floor ~188 us/core is the roofline for this memory-regime problem).

Engine split per 8-batch granule [128p, 8b, 4t, 128d], from measured
per-op rates (microbench.py): DMA ~189 us busy, DVE ~175, ACT ~128,
PE ~33 — DMA-bound as a memory-regime kernel should be.
  DMA   x granule f32 in (2 KB packets, issued from SP); the output
        store is issued from ACT so it never blocks the SP x stream.
  ACT   f32 -> bf16 convert (~4.1 us/granule) + staged exp.
  DVE   scores: bf16 TT-mult against broadcast qk (no STT accumulator —
        its ~460 ns/instr fixed cost killed an earlier version), 4
        strided bf16 halving folds 128->8, segmented tensor_reduce
        8->1 into f32 scores; staged ps-add and L-partial reduce.
  PE    4 bf16 matmuls per batch (stationary x tile, moving e column)
        accumulating T[:,b] PSUM columns; per-block: ones-matmul for L,
        bf16 Wv^T projection, then one DVE STT (proj*(1/L) + bv) and
        the store. Stage sizes ramp 8/16/32 at the stream edges to cut
        pipeline fill/drain.
"""

import numpy as np

TOKEN_DIM = 128
SCALE = TOKEN_DIM ** -0.5
B, S, D = 2048, 512, 128
NCORES = 8
BSH = B // NCORES          # 256 batches per core
TPB = 4                    # tokens per partition per batch (s = 4p + t)
GR = 8                     # batches per granule
NGR = BSH // GR            # 32 granules per core
BLK = 128                  # batches per output block
GPB = BLK // GR            # granules per block (16)
NBLK = BSH // BLK          # 2
SEG = GR * TPB             # 32 score segments per granule
STG = 4                    # granules per exp/L staging group
GSB = STG * GR             # batches per staging group (32)

_CACHE = {}


def _split_multi_waits(nc):
    """The walrus build here rejects instructions carrying more than one
    semaphore wait (limit varies by ISA struct; STT and Drain allow 1).
    Hoist extra waits onto same-engine NoOps placed just before the
    instruction — identical blocking semantics, trivial cost."""
    from concourse import mybir

    n = 0
    for f in nc.m.functions:
        for bb in f.blocks:
            new = []
            for inst in bb.instructions:
                si = inst.sync_info
                if si is not None and si.on_wait and len(si.on_wait) > 1:
                    waits = list(si.on_wait)
                    for w in waits[1:]:
                        n += 1
                        nop = mybir.InstNoOp(
                            name=f"T-wsplit-{n}", engine=inst.engine, ins=[], outs=[]
                        )
                        nop.sync_info = mybir.SyncInfo(on_wait=[w], on_update=[])
                        new.append(nop)
                    inst.sync_info = mybir.SyncInfo(
                        on_wait=[waits[0]], on_update=list(si.on_update or [])
                    )
                new.append(inst)
            bb.instructions = new
    return n


def build_program():
    """Build the per-core Bass program (SPMD across the 8 cores)."""
    import concourse.bass as bass
    import concourse.tile as tile
    from concourse import mybir

    f32 = mybir.dt.float32
    bf16 = mybir.dt.bfloat16
    Exp = mybir.ActivationFunctionType.Exp
    Copy = mybir.ActivationFunctionType.Copy
    Add = mybir.AluOpType.add
    Mult = mybir.AluOpType.mult
    X = mybir.AxisListType.X

    nc = bass.Bass("TRN2", target_bir_lowering=False, debug=False)
    x_d = nc.dram_tensor("x", [BSH, S, D], f32, kind="ExternalInput").ap()
    ps_d = nc.dram_tensor("ps", [128, BSH, TPB], f32, kind="ExternalInput").ap()
    qkr_d = nc.dram_tensor("qkr", [128, TPB * D], bf16, kind="ExternalInput").ap()
    wvt_d = nc.dram_tensor("wvt", [D, D], bf16, kind="ExternalInput").ap()
    bvb_d = nc.dram_tensor("bvb", [128, D], f32, kind="ExternalInput").ap()
    out_d = nc.dram_tensor("out", [BSH, D], f32, kind="ExternalOutput").ap()

    with tile.TileContext(nc) as tc:
        with (
            tc.tile_pool(name="consts", bufs=1) as consts,
            tc.tile_pool(name="xf", bufs=4) as xf_pool,
            tc.tile_pool(name="xb", bufs=6) as xb_pool,
            tc.tile_pool(name="prod", bufs=1) as prod_pool,
            tc.tile_pool(name="f1", bufs=1) as f1_pool,
            tc.tile_pool(name="f2", bufs=1) as f2_pool,
            tc.tile_pool(name="f3", bufs=1) as f3_pool,
            tc.tile_pool(name="f4", bufs=1) as f4_pool,
            tc.tile_pool(name="sc", bufs=2) as sc_pool,
            tc.tile_pool(name="sce", bufs=2) as sce_pool,
            tc.tile_pool(name="e", bufs=2) as e_pool,
            tc.tile_pool(name="P", bufs=2) as P_pool,
            tc.tile_pool(name="tpsum", bufs=2, space="PSUM") as tpsum_pool,
            tc.tile_pool(name="epi_psum", bufs=2, space="PSUM") as epi_psum,
            tc.tile_pool(name="epi", bufs=4) as epi_pool,
        ):
            # first x granule goes out before the const burst (shaves startup)
            prefetch = []
            xfp = xf_pool.tile([128, GR, TPB, D], f32, tag="xfp")
            nc.sync.dma_start(
                xfp[:], x_d[0:GR].rearrange("b (p t) d -> p b t d", t=TPB)
            )
            prefetch.append(xfp)
            qkr_sb = consts.tile([128, TPB * D], bf16)
            nc.sync.dma_start(qkr_sb[:], qkr_d[:])
            bvb_sb = consts.tile([128, D], f32)
            nc.sync.dma_start(bvb_sb[:], bvb_d[:])
            ps_sb = consts.tile([128, BSH, TPB], f32)
            nc.sync.dma_start(ps_sb[:], ps_d[:])
            wvt_sb = consts.tile([D, D], bf16)
            nc.sync.dma_start(wvt_sb[:], wvt_d[:])
            ones_sb = consts.tile([128, 1], f32)
            nc.vector.memset(ones_sb[:], 1.0)

            def qk_bcast(gr):
                return qkr_sb[:].rearrange("p (o f) -> p o f", o=1).broadcast_to(
                    [128, gr, TPB * D]
                )

            # Per-block stage plans: each stage is a list of granule sizes
            # (batches). Small stages at the start of block 0 (DVE gets fed
            # early) and at the end of block 1 (short drain after last DMA).
            # Each block's batch total is BLK=128.
            stage_plan = [
                [[8], [8], [8, 8], [8, 8, 8, 8], [8, 8, 8, 8], [8, 8, 8, 8]],
                [[8, 8, 8, 8], [8, 8, 8, 8], [8, 8, 8, 8], [8, 8], [8], [8]],
            ]
            for blk in range(NBLK):
                Tpsum = tpsum_pool.tile([128, BLK], f32)
                P_blk = P_pool.tile([128, BLK], f32)
                bofs = 0          # batch offset within the block
                for stage in stage_plan[blk]:
                    gsb = sum(stage)
                    st_b0 = bofs
                    sc = sc_pool.tile([128, gsb, TPB], f32)
                    parts = []    # (xb tile, gr, batch offset in stage)
                    sofs = 0
                    for gr in stage:
                        b0 = blk * BLK + bofs
                        seg = gr * TPB
                        if prefetch:
                            xf = prefetch.pop(0)
                        else:
                            xf = xf_pool.tile([128, gr, TPB, D], f32)
                            nc.sync.dma_start(
                                xf[:],
                                x_d[b0 : b0 + gr].rearrange(
                                    "b (p t) d -> p b t d", t=TPB
                                ),
                            )
                        xb = xb_pool.tile([128, gr, TPB, D], bf16)
                        nc.scalar.activation(xb[:], xf[:], Copy)
                        parts.append((xb, gr, sofs))

                        # scores: prod = xb*qk (bf16), fold 128 -> 8, reduce
                        prod = prod_pool.tile([128, seg, D], bf16)
                        nc.vector.tensor_tensor(
                            out=prod[:].rearrange("p s d -> p (s d)").rearrange(
                                "p (b f) -> p b f", b=gr
                            ),
                            in0=xb[:].rearrange("p b t d -> p b (t d)"),
                            in1=qk_bcast(gr),
                            op=Mult,
                        )
                        f1 = f1_pool.tile([128, seg, 64], bf16)
                        nc.vector.tensor_tensor(
                            out=f1[:], in0=prod[:, :, 0:64], in1=prod[:, :, 64:128],
                            op=Add,
                        )
                        f2 = f2_pool.tile([128, seg, 32], bf16)
                        nc.vector.tensor_tensor(
                            out=f2[:], in0=f1[:, :, 0:32], in1=f1[:, :, 32:64], op=Add
                        )
                        f3 = f3_pool.tile([128, seg, 16], bf16)
                        nc.vector.tensor_tensor(
                            out=f3[:], in0=f2[:, :, 0:16], in1=f2[:, :, 16:32], op=Add
                        )
                        f4 = f4_pool.tile([128, seg, 8], bf16)
                        nc.vector.tensor_tensor(
                            out=f4[:], in0=f3[:, :, 0:8], in1=f3[:, :, 8:16], op=Add
                        )
                        nc.vector.tensor_reduce(
                            out=sc[:, sofs : sofs + gr, :].rearrange(
                                "p b t -> p (b t)"
                            ),
                            in_=f4[:], axis=X, op=Add,
                        )
                        sofs += gr
                        bofs += gr
                    # staged softmax prep over this stage's batches
                    sb0 = blk * BLK + st_b0
                    sce = sce_pool.tile([128, gsb, TPB], f32)
                    nc.vector.tensor_tensor(
                        out=sce[:], in0=sc[:], in1=ps_sb[:, sb0 : sb0 + gsb, :], op=Add
                    )
                    e = e_pool.tile([128, gsb, TPB], bf16)
                    nc.scalar.activation(e[:], sce[:], Exp)
                    nc.vector.tensor_reduce(
                        out=P_blk[:, st_b0 : st_b0 + gsb], in_=e[:], axis=X, op=Add
                    )
                    for xb, gr, sofs in parts:
                        for j in range(gr):
                            bcol = st_b0 + sofs + j
                            for t in range(TPB):
                                nc.tensor.matmul(
                                    out=Tpsum[:, bcol : bcol + 1],
                                    lhsT=xb[:, j, t, :],
                                    rhs=e[:, sofs + j, t : t + 1],
                                    start=(t == 0),
                                    stop=(t == TPB - 1),
                                )
                # block epilogue
                Lp = epi_psum.tile([128, 1], f32, tag="Lp")
                nc.tensor.matmul(
                    out=Lp[:], lhsT=P_blk[:], rhs=ones_sb[:], start=True, stop=True
                )
                rcpL = epi_pool.tile([128, 1], f32, tag="rcpL")
                nc.vector.reciprocal(rcpL[:], Lp[:])
                Tsb = epi_pool.tile([128, BLK], bf16, tag="Tsb")
                nc.scalar.activation(Tsb[:], Tpsum[:], Copy)
                proj = epi_psum.tile([128, D], f32, tag="proj")
                nc.tensor.matmul(
                    out=proj[:], lhsT=Tsb[:], rhs=wvt_sb[:], start=True, stop=True
                )
                out_sb = epi_pool.tile([128, D], f32, tag="out_sb")
                nc.vector.scalar_tensor_tensor(
                    out=out_sb[:], in0=proj[:], scalar=rcpL[:], in1=bvb_sb[:],
                    op0=Mult, op1=Add,
                )
                # issue from ACT so the SP engine's DMA stream (next block's x
                # granules) is not serialized behind the epilogue chain
                nc.scalar.dma_start(out_d[blk * BLK : (blk + 1) * BLK, :], out_sb[:])

    _split_multi_waits(nc)
    return nc


def prepare_inputs(input_features, positions, mask, query, Wk, bk, Wv, bv, Wp, bp):
    """Host-side prep: shard along batch, replicate/fold the small weights."""
    import ml_dtypes

    q = np.asarray(query, np.float32)[0]
    qk = (q @ np.asarray(Wk, np.float32)) * SCALE           # [D]
    qp = (q @ np.asarray(Wp, np.float32)) * SCALE           # [4]
    qkr = np.ascontiguousarray(
        np.broadcast_to(
            np.tile(qk, TPB)[None, :].astype(ml_dtypes.bfloat16), (128, TPB * D)
        )
    )
    wvt = np.ascontiguousarray(np.asarray(Wv, np.float32).T.astype(ml_dtypes.bfloat16))
    bvb = np.ascontiguousarray(
        np.broadcast_to(np.asarray(bv, np.float32)[None, :], (128, D))
    )

    # ps[b, s] = pos . qp with masked tokens forced to -1e30 so their
    # softmax weight underflows to exactly 0. Packed as [128 p, B, 4 t]
    # matching the device's s = 4p + t token scramble.
    ps = np.asarray(positions, np.float32) @ qp              # [B, S]
    m = np.asarray(mask, bool)
    if not m.all():
        ps = np.where(m, ps, np.float32(-1e30))
    ps = np.ascontiguousarray(
        ps.reshape(B, 128, TPB).transpose(1, 0, 2), np.float32
    )

    x = np.asarray(input_features, np.float32)
    in_maps = []
    for c in range(NCORES):
        in_maps.append(
            {
                "x": x[c * BSH : (c + 1) * BSH],
                "ps": np.ascontiguousarray(ps[:, c * BSH : (c + 1) * BSH]),
                "qkr": qkr,
                "wvt": wvt,
                "bvb": bvb,
            }
        )
    return in_maps


def kernel(input_features, positions, mask, query, Wk, bk, Wv, bv, Wp, bp):
    from concourse.bass_utils import run_bass_kernel_spmd

    if "nc" not in _CACHE:
        _CACHE["nc"] = build_program()
    nc = _CACHE["nc"]
    in_maps = prepare_inputs(
        input_features, positions, mask, query, Wk, bk, Wv, bv, Wp, bp
    )
    res = run_bass_kernel_spmd(nc, in_maps, list(range(NCORES)))
    return np.concatenate([res.results[c]["out"] for c in range(NCORES)], axis=0)
